# revision 14
# baseline (speedup 1.0000x reference)
"""Trainium2 Bass kernel for a 2-layer causal transformer decoder with a
ragged repeat-expand prologue.

Distribution: DP=2 over batch x TP=4 over heads / FFN hidden dim
(cores 0-3 -> batch 0, cores 4-7 -> batch 1).  Within a TP group the
residual stream is sequence-sharded; each AllGather / ReduceScatter is
split into NCHUNK chunks that pipeline with the surrounding matmuls so
collective latency is hidden.  Row ownership is strided by row-tile so
that a ReduceScatter chunk's output lands exactly on its owner.

All matmuls run in fp16 (weights are folded with the RMS-norm gains and
cast on the host); softmax and normalizations run in fp32.
"""

import numpy as np

import concourse.bass as bass
import concourse.mybir as mybir
import concourse.tile as tile
from concourse import bacc
from concourse import bass_utils
from concourse.masks import make_identity

P = 128
EPS = 1e-5
NCORES = 8
TP = 4
DP = 2
GROUPS = [[0, 1, 2, 3], [4, 5, 6, 7]]

F16 = mybir.dt.float16
F32 = mybir.dt.float32
I32 = mybir.dt.int32
AF = mybir.ActivationFunctionType
ALU = mybir.AluOpType

FULL_CFG = dict(B=2, S=2048, D=1024, K=256, H=16, HD=64, F=4096, L=2)


def _derive(cfg):
    d = dict(cfg)
    d["LH"] = cfg["H"] // TP               # local heads
    d["OC"] = d["LH"] * cfg["HD"]          # local attention channels
    assert d["OC"] % P == 0
    d["NOPT"] = d["OC"] // P               # q/k/v/o channel partition-tiles
    d["HPP"] = P // cfg["HD"]              # heads per partition-tile
    d["OWN"] = cfg["S"] // TP              # rows owned by this core
    assert d["OWN"] % P == 0
    d["NRT"] = d["OWN"] // P               # own row tiles
    d["NRA"] = cfg["S"] // P               # all row tiles
    d["KTD"] = cfg["D"] // P               # contraction tiles over D
    d["FL"] = cfg["F"] // TP               # local FFN width
    d["KTF"] = d["FL"] // P                # contraction tiles over FL
    d["NCHUNK"] = d["NRT"]                 # collective chunks (R = 1)
    d["R"] = d["NRT"] // d["NCHUNK"]       # own row-tiles per chunk
    d["CW"] = TP * d["R"] * P              # global rows per chunk
    d["QC"] = d["CW"]                      # attention q chunk == chunk width
    assert d["QC"] <= 512
    d["NQC"] = cfg["S"] // d["QC"]
    assert d["NQC"] == d["NCHUNK"]
    assert cfg["HD"] == 64
    return d


def build_nc(cfg):
    """Builds the SPMD Bass program (identical on all 8 cores)."""
    c = _derive(cfg)
    B, S, D, K, H, HD, F, L = (c[k] for k in ("B", "S", "D", "K", "H", "HD", "F", "L"))
    LH, OC, NOPT, HPP = c["LH"], c["OC"], c["NOPT"], c["HPP"]
    OWN, NRT, NRA, KTD = c["OWN"], c["NRT"], c["NRA"], c["KTD"]
    FL, KTF, QC, NQC = c["FL"], c["KTF"], c["QC"], c["NQC"]
    NCHUNK, R, CW = c["NCHUNK"], c["R"], c["CW"]
    RPC = CW // P                          # global row-tiles per chunk
    ISCALE = float(HD) ** -0.5

    nc = bacc.Bacc("TRN2", target_bir_lowering=False, debug=False,
                   num_devices=NCORES)

    # ---- per-core external inputs (host pre-sharded / pre-laid-out) ----
    reps_d = nc.dram_tensor("reps", [K + 1, D], F32, kind="ExternalInput")
    segq_d = nc.dram_tensor("segq", [OWN, 1], I32, kind="ExternalInput")
    resid_d = nc.dram_tensor("resid", [OWN, D], F32, kind="ExternalInput")
    cosb_d = nc.dram_tensor("cosb", [P, S], F16, kind="ExternalInput")
    sgnsin_d = nc.dram_tensor("sgnsin", [P, S], F16, kind="ExternalInput")
    fnormb_d = nc.dram_tensor("fnormb", [P, D], F16, kind="ExternalInput")
    masks_d = nc.dram_tensor("masks", [P, (CW // P) * QC], F16, kind="ExternalInput")
    wq_d = nc.dram_tensor("wq", [L, P, KTD * OC], F16, kind="ExternalInput")
    wk_d = nc.dram_tensor("wk", [L, P, KTD * OC], F16, kind="ExternalInput")
    wv_d = nc.dram_tensor("wv", [L, P, KTD * OC], F16, kind="ExternalInput")
    wo_d = nc.dram_tensor("wo", [L, P, NOPT * D], F16, kind="ExternalInput")
    wg_d = nc.dram_tensor("wg", [L, P, KTD * FL], F16, kind="ExternalInput")
    wu_d = nc.dram_tensor("wu", [L, P, KTD * FL], F16, kind="ExternalInput")
    wd_d = nc.dram_tensor("wd", [L, P, KTF * D], F16, kind="ExternalInput")
    out_d = nc.dram_tensor("out", [OWN, D], F32, kind="ExternalOutput")

    d_chunks = [(s, min(512, D - s)) for s in range(0, D, 512)]

    with tile.TileContext(nc) as tc:
        with tc.tile_pool(name="const", bufs=1) as cpool, \
             tc.tile_pool(name="xres", bufs=1) as xpool, \
             tc.tile_pool(name="ht", bufs=1) as hpool, \
             tc.tile_pool(name="big", bufs=1) as bigpool, \
             tc.tile_pool(name="vv", bufs=1) as vpool, \
             tc.tile_pool(name="wts", bufs=1) as wpool, \
             tc.tile_pool(name="scr", bufs=2) as spool, \
             tc.tile_pool(name="psum", bufs=1, space="PSUM") as ppool, \
             tc.tile_pool(name="dram", bufs=1, space="DRAM") as dpool:

            # ---- constants ----
            ones64 = cpool.tile([1, 64], F32, name="ones64")
            nc.vector.memset(ones64[:], 1.0)
            zb = cpool.tile([P, 1], F32, name="zb")
            nc.vector.memset(zb[:], 0.0)
            eb = cpool.tile([P, 1], F32, name="eb")
            nc.vector.memset(eb[:], EPS)
            cosb = cpool.tile([P, S], F16, name="cosb")
            nc.sync.dma_start(cosb[:], cosb_d[:])
            sgnsin = cpool.tile([P, S], F16, name="sgnsin")
            nc.sync.dma_start(sgnsin[:], sgnsin_d[:])
            fnormb = cpool.tile([P, D], F16, name="fnormb")
            nc.sync.dma_start(fnormb[:], fnormb_d[:])
            maskt = cpool.tile([P, RPC * QC], F16, name="maskt")
            nc.sync.dma_start(maskt[:], masks_d[:])

            # ---- ragged expand: x_own = reps[seg] + resid (local order) ----
            x_own = []
            for i in range(NRT):
                xt = xpool.tile([P, D], F32, name=f"x{i}", tag=f"x{i}")
                idx = spool.tile([P, 1], I32, name="idx", tag="idx", bufs=2)
                nc.sync.dma_start(idx[:], segq_d[i * P:(i + 1) * P, :])
                nc.gpsimd.indirect_dma_start(
                    out=xt[:], out_offset=None, in_=reps_d[:],
                    in_offset=bass.IndirectOffsetOnAxis(ap=idx[:, :1], axis=0))
                res = spool.tile([P, D], F32, name="res", tag="res", bufs=1)
                nc.sync.dma_start(res[:], resid_d[i * P:(i + 1) * P, :])
                nc.vector.tensor_add(xt[:], xt[:], res[:])
                x_own.append(xt)

            def rms_inv(src_ap, scratch_ap):
                ss = spool.tile([P, 1], F32, name="ss", tag="ss", bufs=2)
                nc.scalar.activation(scratch_ap, src_ap, AF.Square,
                                     bias=zb[:, :1], accum_out=ss[:])
                st = spool.tile([P, 1], F32, name="st", tag="st", bufs=2)
                nc.scalar.activation(st[:], ss[:], AF.Sqrt, scale=1.0 / D,
                                     bias=eb[:, :1])
                inv = spool.tile([P, 1], F32, name="inv", tag="inv", bufs=2)
                nc.vector.reciprocal_approx_fast(inv[:], st[:])
                return inv

            def alloc_ht(phase):
                return [hpool.tile([P, S], F16, name=f"ht{kt}_{phase}",
                                   tag=f"ht{kt}") for kt in range(KTD)]

            def ln_transpose_ag(phase, cc, ht):
                """LN chunk cc of x_own, transpose, AllGather, load into ht."""
                agin = dpool.tile([R * P, D], F16, name=f"agin_{phase}_{cc}",
                                  tag="agin", bufs=2 * NCHUNK)
                for j in range(R):
                    i = cc * R + j
                    h = spool.tile([P, D], F16, name="h", tag="h", bufs=2)
                    inv = rms_inv(x_own[i][:], h[:])
                    nc.vector.tensor_scalar_mul(h[:], x_own[i][:], inv[:, :1])
                    nc.sync.dma_start(agin[j * P:(j + 1) * P, :], h[:])
                agout = dpool.tile([TP, R * P, D], F16,
                                   name=f"agout_{phase}_{cc}", tag="agout",
                                   bufs=2 * NCHUNK)
                nc.gpsimd.collective_compute(
                    "AllGather", ALU.bypass, replica_groups=GROUPS,
                    ins=[agin[:]], outs=[agout[:]])
                for kt in range(KTD):
                    for r in range(TP):
                        for j in range(R):
                            col = ((cc * TP + r) * R + j) * P
                            nc.sync.dma_start_transpose(
                                ht[kt][:, col:col + P],
                                agout[r, j * P:(j + 1) * P,
                                      kt * P:(kt + 1) * P])

            def rs_chunk_add(rsin, phase, cc):
                """ReduceScatter chunk cc and add into own rows."""
                rsout = dpool.tile([R * P, D], F16, name=f"rso_{phase}_{cc}",
                                   tag="rsout", bufs=2 * NCHUNK)
                nc.gpsimd.collective_compute(
                    "ReduceScatter", ALU.add, replica_groups=GROUPS,
                    ins=[rsin[:]], outs=[rsout[:]])
                for j in range(R):
                    i = cc * R + j
                    rsl = spool.tile([P, D], F16, name="rsl", tag="rsl", bufs=2)
                    nc.sync.dma_start(rsl[:], rsout[j * P:(j + 1) * P, :])
                    nc.vector.tensor_add(x_own[i][:], x_own[i][:], rsl[:])

            # ---- initial AllGather for layer-0 attention ----
            ht = alloc_ht("a0")
            for cc in range(NCHUNK):
                ln_transpose_ag("a0", cc, ht)

            for l in range(L):
                # ---- attention weights ----
                wq = wpool.tile([P, KTD * OC], F16, name="wq", tag="wq")
                nc.sync.dma_start(wq[:], wq_d[l])
                wk = wpool.tile([P, KTD * OC], F16, name="wk", tag="wk")
                nc.sync.dma_start(wk[:], wk_d[l])
                wv = wpool.tile([P, KTD * OC], F16, name="wv", tag="wv")
                nc.sync.dma_start(wv[:], wv_d[l])
                wo = wpool.tile([P, NOPT * D], F16, name="wo", tag="wo")
                nc.sync.dma_start(wo[:], wo_d[l])

                # ---- Q/K (channel-major + RoPE) and V (row-major) ----
                qt = [bigpool.tile([P, S], F16, name=f"q{m}", tag=f"big{m}")
                      for m in range(NOPT)]
                ktt = [bigpool.tile([P, S], F16, name=f"k{m}",
                                    tag=f"big{NOPT + m}")
                       for m in range(NOPT)]
                vt = [vpool.tile([P, LH * 65], F16, name=f"v{rt}", tag=f"v{rt}")
                      for rt in range(NRA)]

                for sc in range(NQC):
                    ns = sc * QC
                    for wsb, outs in ((wq, qt), (wk, ktt)):
                        for m in range(NOPT):
                            pq = ppool.tile([P, QC], F32, name="pq", tag="proj",
                                            bufs=2, space="PSUM")
                            for kt in range(KTD):
                                nc.tensor.matmul(
                                    pq[:],
                                    lhsT=wsb[:, kt * OC + m * P:
                                             kt * OC + (m + 1) * P],
                                    rhs=ht[kt][:, ns:ns + QC],
                                    start=(kt == 0), stop=(kt == KTD - 1))
                            nc.scalar.copy(outs[m][:, ns:ns + QC], pq[:])
                        # RoPE on this column chunk
                        for t in outs:
                            rot = spool.tile([P, QC], F16, name="rot",
                                             tag="rot", bufs=2)
                            for blk in range(HPP):
                                o = blk * HD
                                hh = HD // 2
                                nc.sync.dma_start(rot[o:o + hh, :],
                                                  t[o + hh:o + HD, ns:ns + QC])
                                nc.sync.dma_start(rot[o + hh:o + HD, :],
                                                  t[o:o + hh, ns:ns + QC])
                            nc.vector.tensor_mul(t[:, ns:ns + QC],
                                                 t[:, ns:ns + QC],
                                                 cosb[:, ns:ns + QC])
                            nc.vector.tensor_mul(rot[:], rot[:],
                                                 sgnsin[:, ns:ns + QC])
                            nc.vector.tensor_add(t[:, ns:ns + QC],
                                                 t[:, ns:ns + QC], rot[:])
                    # V for the row-tiles of this chunk
                    for rt in range(sc * RPC, (sc + 1) * RPC):
                        v = vt[rt]
                        v3 = v[:].rearrange("p (h c) -> p h c", c=65)
                        nc.vector.memset(v3[:, :, 64:65], 1.0)
                        pv = ppool.tile([P, 512], F32, name="pv", tag="proj",
                                        bufs=2, space="PSUM")
                        for kt in range(KTD):
                            nc.tensor.matmul(
                                pv[:, :OC],
                                lhsT=ht[kt][:, rt * P:(rt + 1) * P],
                                rhs=wv[:, kt * OC:(kt + 1) * OC],
                                start=(kt == 0), stop=(kt == KTD - 1))
                        nc.scalar.copy(
                            v3[:, :, :64],
                            pv[:, :OC].rearrange("p (h c) -> p h c", c=64))

                # ---- flash-style causal attention + Wo + RS, per q-chunk ----
                ot = [bigpool.tile([P, S], F16, name=f"o{m}",
                                   tag=f"big{2 * NOPT + m}")
                      for m in range(NOPT)]
                htn = alloc_ht(f"f{l}")       # next-phase hT (FFN)
                for qc in range(NQC):
                    q0 = qc * QC
                    nkt = (q0 + QC) // P
                    sums = spool.tile([1, LH * QC], F32, name="sums",
                                      tag="sums", bufs=1)
                    for h in range(LH):
                        hp, ho = divmod(h * HD, P)
                        po = ppool.tile([P, QC], F32, name="po", tag="ot",
                                        bufs=2, space="PSUM")
                        for kt in range(nkt):
                            ps = ppool.tile([P, QC], F32, name="ps", tag="sc",
                                            bufs=3, space="PSUM")
                            nc.tensor.matmul(
                                ps[:],
                                lhsT=ktt[hp][ho:ho + HD, kt * P:(kt + 1) * P],
                                rhs=qt[hp][ho:ho + HD, q0:q0 + QC],
                                start=True, stop=True)
                            ex = spool.tile([P, QC], F16, name="ex", tag="ex",
                                            bufs=3)
                            nc.scalar.activation(ex[:], ps[:], AF.Exp,
                                                 bias=zb[:, :1], scale=ISCALE)
                            dd = kt - q0 // P
                            if dd >= 0:
                                nc.vector.tensor_mul(
                                    ex[:], ex[:],
                                    maskt[:, dd * QC:(dd + 1) * QC])
                            nc.tensor.matmul(
                                po[0:65, :],
                                lhsT=vt[kt][:, h * 65:(h + 1) * 65],
                                rhs=ex[:],
                                start=(kt == 0), stop=(kt == nkt - 1))
                        # unnormalized o' and the denominator row
                        nc.scalar.copy(ot[hp][ho:ho + HD, q0:q0 + QC],
                                       po[0:64, :])
                        nc.scalar.copy(sums[0:1, h * QC:(h + 1) * QC], po[64:65, :])
                    nc.vector.reciprocal_approx_fast(sums[:], sums[:])
                    for h in range(LH):
                        hp, ho = divmod(h * HD, P)
                        pb = ppool.tile([64, QC], F32, name="pb", tag="ot",
                                        bufs=2, space="PSUM")
                        nc.tensor.matmul(pb[:], lhsT=ones64[0:1, :],
                                         rhs=sums[0:1, h * QC:(h + 1) * QC],
                                         start=True, stop=True)
                        nc.vector.tensor_mul(ot[hp][ho:ho + HD, q0:q0 + QC],
                                             ot[hp][ho:ho + HD, q0:q0 + QC],
                                             pb[:])
                    # Wo for this chunk's row-tiles -> RS -> x += ; then the
                    # FFN-phase LN/transpose/AllGather for the same chunk.
                    rsin = dpool.tile([CW, D], F16, name=f"rsi_a{l}_{qc}",
                                      tag="rsin", bufs=2 * NCHUNK)
                    for rt in range(qc * RPC, (qc + 1) * RPC):
                        ow = spool.tile([P, D], F16, name="ow", tag="ow",
                                        bufs=2)
                        for (ds, dl) in d_chunks:
                            pw = ppool.tile([P, 512], F32, name="pw",
                                            tag="proj", bufs=2, space="PSUM")
                            for n in range(NOPT):
                                nc.tensor.matmul(
                                    pw[:, :dl],
                                    lhsT=ot[n][:, rt * P:(rt + 1) * P],
                                    rhs=wo[:, n * D + ds: n * D + ds + dl],
                                    start=(n == 0), stop=(n == NOPT - 1))
                            nc.vector.tensor_copy(ow[:, ds:ds + dl],
                                                  pw[:, :dl])
                        nc.sync.dma_start(
                            rsin[(rt - qc * RPC) * P:(rt - qc * RPC + 1) * P, :],
                            ow[:])
                    rs_chunk_add(rsin, f"a{l}", qc)
                    ln_transpose_ag(f"f{l}", qc, htn)
                ht = htn

                # ---- FFN ----
                wg = wpool.tile([P, KTD * FL], F16, name="wg", tag="wg")
                nc.sync.dma_start(wg[:], wg_d[l])
                wu = wpool.tile([P, KTD * FL], F16, name="wu", tag="wu")
                nc.sync.dma_start(wu[:], wu_d[l])
                wd = wpool.tile([P, KTF * D], F16, name="wd", tag="wd")
                nc.sync.dma_start(wd[:], wd_d[l])

                at = [bigpool.tile([P, S], F16, name=f"a{fm}", tag=f"big{fm}")
                      for fm in range(KTF)]
                if l < L - 1:
                    htn = alloc_ht(f"a{l + 1}")
                for sc in range(NQC):
                    ns = sc * QC
                    for fm in range(KTF):
                        pg = ppool.tile([P, QC], F32, name="pg", tag="proj",
                                        bufs=2, space="PSUM")
                        for kt in range(KTD):
                            nc.tensor.matmul(
                                pg[:],
                                lhsT=wg[:, kt * FL + fm * P:
                                        kt * FL + (fm + 1) * P],
                                rhs=ht[kt][:, ns:ns + QC],
                                start=(kt == 0), stop=(kt == KTD - 1))
                        pu = ppool.tile([P, QC], F32, name="pu", tag="sc",
                                        bufs=3, space="PSUM")
                        for kt in range(KTD):
                            nc.tensor.matmul(
                                pu[:],
                                lhsT=wu[:, kt * FL + fm * P:
                                        kt * FL + (fm + 1) * P],
                                rhs=ht[kt][:, ns:ns + QC],
                                start=(kt == 0), stop=(kt == KTD - 1))
                        sg = spool.tile([P, QC], F16, name="sg", tag="sg",
                                        bufs=2)
                        nc.scalar.activation(sg[:], pg[:], AF.Sigmoid,
                                             bias=zb[:, :1])
                        nc.vector.tensor_mul(sg[:], sg[:], pg[:])
                        nc.vector.tensor_mul(at[fm][:, ns:ns + QC], sg[:],
                                             pu[:])
                    # Wd for this chunk -> RS -> x += ; then next-phase AG
                    rsin2 = dpool.tile([CW, D], F16, name=f"rsi_f{l}_{sc}",
                                       tag="rsin", bufs=2 * NCHUNK)
                    for rt in range(sc * RPC, (sc + 1) * RPC):
                        dw = spool.tile([P, D], F16, name="dw", tag="ow",
                                        bufs=2)
                        for (ds, dl) in d_chunks:
                            pd = ppool.tile([P, 512], F32, name="pd",
                                            tag="proj", bufs=2, space="PSUM")
                            for kt in range(KTF):
                                nc.tensor.matmul(
                                    pd[:, :dl],
                                    lhsT=at[kt][:, rt * P:(rt + 1) * P],
                                    rhs=wd[:, kt * D + ds: kt * D + ds + dl],
                                    start=(kt == 0), stop=(kt == KTF - 1))
                            nc.vector.tensor_copy(dw[:, ds:ds + dl],
                                                  pd[:, :dl])
                        nc.sync.dma_start(
                            rsin2[(rt - sc * RPC) * P:
                                  (rt - sc * RPC + 1) * P, :],
                            dw[:])
                    rs_chunk_add(rsin2, f"f{l}", sc)
                    if l < L - 1:
                        ln_transpose_ag(f"a{l + 1}", sc, htn)
                    else:
                        # final RMS norm * fnorm for this chunk's rows
                        for j in range(R):
                            i = sc * R + j
                            fo = spool.tile([P, D], F32, name="fo", tag="fo",
                                            bufs=1)
                            inv = rms_inv(x_own[i][:], fo[:])
                            nc.vector.tensor_scalar_mul(fo[:], x_own[i][:],
                                                        inv[:, :1])
                            nc.vector.tensor_mul(fo[:], fo[:], fnormb[:])
                            nc.sync.dma_start(out_d[i * P:(i + 1) * P, :],
                                              fo[:])
                if l < L - 1:
                    ht = htn

    nc.compile()
    return nc


# --------------------------------------------------------------------------
# host-side input preparation
# --------------------------------------------------------------------------

def _own_rowtiles(c, tp):
    """Global row-tile indices owned by rank tp, in local order."""
    return [(cc * TP + tp) * c["R"] + j
            for cc in range(c["NCHUNK"]) for j in range(c["R"])]


def make_in_maps(cfg, inputs):
    c = _derive(cfg)
    B, S, D, K, H, HD, F, L = (c[k] for k in ("B", "S", "D", "K", "H", "HD", "F", "L"))
    LH, OC, NOPT, OWN = c["LH"], c["OC"], c["NOPT"], c["OWN"]
    KTD, FL, KTF = c["KTD"], c["FL"], c["KTF"]

    gi = {k: np.asarray(v) for k, v in inputs.items() if k != "seq_len"}
    x_processed = gi["x_processed"].astype(np.float32)
    boundaries = gi["boundaries"].astype(np.int64)
    counts = gi["counts"].astype(np.int64)
    x_residual = gi["x_residual"].astype(np.float32)
    cos = gi["cos"].astype(np.float32)
    sin = gi["sin"].astype(np.float32)
    start_emb = gi["start_emb"].astype(np.float32)
    ln1 = gi["ln1"].astype(np.float32)
    ln2 = gi["ln2"].astype(np.float32)
    fnorm = gi["fnorm"].astype(np.float32)
    Wq, Wk, Wv, Wo = (gi[k].astype(np.float32) for k in ("Wq", "Wk", "Wv", "Wo"))
    Wg, Wu, Wd = (gi[k].astype(np.float32) for k in ("Wg", "Wu", "Wd"))

    # segment index per position (searchsorted over masked boundaries)
    seg = np.empty((B, S), np.int32)
    for b in range(B):
        bnd = np.where(np.arange(K) < counts[b], boundaries[b], S)
        seg[b] = np.searchsorted(bnd, np.arange(S), side="left").astype(np.int32)

    pidx = np.arange(P)
    cosb = cos.T[pidx % HD].astype(np.float16)                       # [P, S]
    sgn = np.where((pidx % HD) < HD // 2, -1.0, 1.0)[:, None]
    sgnsin = (sin.T[pidx % HD] * sgn).astype(np.float16)             # [P, S]
    fnormb = np.broadcast_to(fnorm, (P, D)).astype(np.float16).copy()
    QCc, RPCc = c["QC"], c["CW"] // P
    kk = np.arange(P)[:, None]
    qq = np.arange(QCc)[None, :]
    masks = np.concatenate(
        [(dd * P + kk <= qq).astype(np.float16) for dd in range(RPCc)],
        axis=1)

    def kt_layout(w):      # [D or FL, C] -> [P, KT*C]
        n, cdim = w.shape
        return np.ascontiguousarray(
            w.reshape(n // P, P, cdim).transpose(1, 0, 2).reshape(P, -1)
        ).astype(np.float16)

    in_maps = []
    for cid in range(NCORES):
        b, tp = divmod(cid, TP)
        gts = _own_rowtiles(c, tp)
        rows = np.concatenate([np.arange(g * P, (g + 1) * P) for g in gts])
        hb = tp * OC
        fb = tp * FL
        reps = np.concatenate([start_emb[None], x_processed[b]], axis=0)
        m = dict(
            reps=np.ascontiguousarray(reps, np.float32),
            segq=np.ascontiguousarray(seg[b, rows].reshape(OWN, 1)),
            resid=np.ascontiguousarray(x_residual[b, rows]),
            cosb=cosb, sgnsin=sgnsin, fnormb=fnormb, masks=masks,
            wq=np.stack([kt_layout(ln1[l][:, None] * Wq[l][:, hb:hb + OC])
                         for l in range(L)]),
            wk=np.stack([kt_layout(ln1[l][:, None] * Wk[l][:, hb:hb + OC])
                         for l in range(L)]),
            wv=np.stack([kt_layout(ln1[l][:, None] * Wv[l][:, hb:hb + OC])
                         for l in range(L)]),
            wo=np.stack([kt_layout(Wo[l][hb:hb + OC, :]) for l in range(L)]),
            wg=np.stack([kt_layout(ln2[l][:, None] * Wg[l][:, fb:fb + FL])
                         for l in range(L)]),
            wu=np.stack([kt_layout(ln2[l][:, None] * Wu[l][:, fb:fb + FL])
                         for l in range(L)]),
            wd=np.stack([kt_layout(Wd[l][fb:fb + FL, :]) for l in range(L)]),
        )
        in_maps.append(m)
    return in_maps


def assemble_output(cfg, results):
    c = _derive(cfg)
    B, S, D, OWN = c["B"], c["S"], c["D"], c["OWN"]
    out = np.empty((B, S, D), np.float32)
    for cid in range(NCORES):
        b, tp = divmod(cid, TP)
        gts = _own_rowtiles(c, tp)
        r = results[cid]["out"]
        for i, g in enumerate(gts):
            out[b, g * P:(g + 1) * P] = r[i * P:(i + 1) * P]
    return out


_NC_CACHE = {}


def _get_nc(cfg):
    key = tuple(sorted(cfg.items()))
    if key not in _NC_CACHE:
        _NC_CACHE[key] = build_nc(cfg)
    return _NC_CACHE[key]


def kernel(**inputs) -> np.ndarray:
    cfg = FULL_CFG
    nc = _get_nc(cfg)
    in_maps = make_in_maps(cfg, inputs)
    res = bass_utils.run_bass_kernel_spmd(nc, in_maps,
                                          core_ids=list(range(NCORES)))
    return assemble_output(cfg, res.results)


# revision 15
# speedup vs baseline: 1.3327x; 1.3327x over previous
"""Trainium2 Bass kernel for a 2-layer causal transformer decoder with a
ragged repeat-expand prologue.

Distribution: DP=2 over batch x TP=4 over heads / FFN hidden dim
(cores 0-3 -> batch 0, cores 4-7 -> batch 1).  Within a TP group the
residual stream is sequence-sharded; each AllGather / ReduceScatter is
split into NCHUNK chunks that pipeline with the surrounding matmuls so
collective latency is hidden.  Row ownership is strided by row-tile so
that a ReduceScatter chunk's output lands exactly on its owner.

All matmuls run in fp16 (weights are folded with the RMS-norm gains and
cast on the host); softmax and normalizations run in fp32.
"""

import numpy as np

import concourse.bass as bass
import concourse.mybir as mybir
import concourse.tile as tile
from concourse import bacc
from concourse import bass_utils
from concourse.masks import make_identity

P = 128
EPS = 1e-5
NCORES = 8
TP = 4
DP = 2
GROUPS = [[0, 1, 2, 3], [4, 5, 6, 7]]

F16 = mybir.dt.float16
F32 = mybir.dt.float32
I32 = mybir.dt.int32
AF = mybir.ActivationFunctionType
ALU = mybir.AluOpType

FULL_CFG = dict(B=2, S=2048, D=1024, K=256, H=16, HD=64, F=4096, L=2)


def _derive(cfg):
    d = dict(cfg)
    d["LH"] = cfg["H"] // TP               # local heads
    d["OC"] = d["LH"] * cfg["HD"]          # local attention channels
    assert d["OC"] % P == 0
    d["NOPT"] = d["OC"] // P               # q/k/v/o channel partition-tiles
    d["HPP"] = P // cfg["HD"]              # heads per partition-tile
    d["OWN"] = cfg["S"] // TP              # rows owned by this core
    assert d["OWN"] % P == 0
    d["NRT"] = d["OWN"] // P               # own row tiles
    d["NRA"] = cfg["S"] // P               # all row tiles
    d["KTD"] = cfg["D"] // P               # contraction tiles over D
    d["FL"] = cfg["F"] // TP               # local FFN width
    d["KTF"] = d["FL"] // P                # contraction tiles over FL
    d["NCHUNK"] = d["NRT"]                 # collective chunks (R = 1)
    d["R"] = d["NRT"] // d["NCHUNK"]       # own row-tiles per chunk
    d["CW"] = TP * d["R"] * P              # global rows per chunk
    d["QC"] = d["CW"]                      # attention q chunk == chunk width
    assert d["QC"] <= 512
    d["NQC"] = cfg["S"] // d["QC"]
    assert d["NQC"] == d["NCHUNK"]
    assert cfg["HD"] == 64
    return d


def build_nc(cfg):
    """Builds the SPMD Bass program (identical on all 8 cores)."""
    c = _derive(cfg)
    B, S, D, K, H, HD, F, L = (c[k] for k in ("B", "S", "D", "K", "H", "HD", "F", "L"))
    LH, OC, NOPT, HPP = c["LH"], c["OC"], c["NOPT"], c["HPP"]
    OWN, NRT, NRA, KTD = c["OWN"], c["NRT"], c["NRA"], c["KTD"]
    FL, KTF, QC, NQC = c["FL"], c["KTF"], c["QC"], c["NQC"]
    NCHUNK, R, CW = c["NCHUNK"], c["R"], c["CW"]
    RPC = CW // P                          # global row-tiles per chunk
    ISCALE = float(HD) ** -0.5

    nc = bacc.Bacc("TRN2", target_bir_lowering=False, debug=False,
                   num_devices=NCORES)

    # ---- per-core external inputs (host pre-sharded / pre-laid-out) ----
    reps_d = nc.dram_tensor("reps", [K + 1, D], F32, kind="ExternalInput")
    segq_d = nc.dram_tensor("segq", [OWN, 1], I32, kind="ExternalInput")
    resid_d = nc.dram_tensor("resid", [OWN, D], F32, kind="ExternalInput")
    cosb_d = nc.dram_tensor("cosb", [P, S], F16, kind="ExternalInput")
    sgnsin_d = nc.dram_tensor("sgnsin", [P, S], F16, kind="ExternalInput")
    fnormb_d = nc.dram_tensor("fnormb", [P, D], F16, kind="ExternalInput")
    masks_d = nc.dram_tensor("masks", [P, (CW // P) * QC], F16, kind="ExternalInput")
    wq_d = nc.dram_tensor("wq", [L, P, KTD * OC], F16, kind="ExternalInput")
    wk_d = nc.dram_tensor("wk", [L, P, KTD * OC], F16, kind="ExternalInput")
    wv_d = nc.dram_tensor("wv", [L, P, KTD * OC], F16, kind="ExternalInput")
    wo_d = nc.dram_tensor("wo", [L, P, NOPT * D], F16, kind="ExternalInput")
    wg_d = nc.dram_tensor("wg", [L, P, KTD * FL], F16, kind="ExternalInput")
    wu_d = nc.dram_tensor("wu", [L, P, KTD * FL], F16, kind="ExternalInput")
    wd_d = nc.dram_tensor("wd", [L, P, KTF * D], F16, kind="ExternalInput")
    out_d = nc.dram_tensor("out", [OWN, D], F32, kind="ExternalOutput")

    d_chunks = [(s, min(512, D - s)) for s in range(0, D, 512)]

    with tile.TileContext(nc) as tc:
        with tc.tile_pool(name="const", bufs=1) as cpool, \
             tc.tile_pool(name="xres", bufs=1) as xpool, \
             tc.tile_pool(name="ht", bufs=1) as hpool, \
             tc.tile_pool(name="big", bufs=1) as bigpool, \
             tc.tile_pool(name="vv", bufs=1) as vpool, \
             tc.tile_pool(name="wts", bufs=1) as wpool, \
             tc.tile_pool(name="scr", bufs=2) as spool, \
             tc.tile_pool(name="psum", bufs=1, space="PSUM") as ppool, \
             tc.tile_pool(name="dram", bufs=1, space="DRAM") as dpool:

            # ---- constants ----
            ones64 = cpool.tile([1, 64], F32, name="ones64")
            nc.vector.memset(ones64[:], 1.0)
            zb = cpool.tile([P, 1], F32, name="zb")
            nc.vector.memset(zb[:], 0.0)
            eb = cpool.tile([P, 1], F32, name="eb")
            nc.vector.memset(eb[:], EPS)
            cosb = cpool.tile([P, S], F16, name="cosb")
            nc.sync.dma_start(cosb[:], cosb_d[:])
            sgnsin = cpool.tile([P, S], F16, name="sgnsin")
            nc.sync.dma_start(sgnsin[:], sgnsin_d[:])
            fnormb = cpool.tile([P, D], F16, name="fnormb")
            nc.sync.dma_start(fnormb[:], fnormb_d[:])
            maskt = cpool.tile([P, RPC * QC], F16, name="maskt")
            nc.sync.dma_start(maskt[:], masks_d[:])

            # ---- ragged expand: x_own = reps[seg] + resid (local order) ----
            x_own = []
            for i in range(NRT):
                xt = xpool.tile([P, D], F32, name=f"x{i}", tag=f"x{i}")
                idx = spool.tile([P, 1], I32, name="idx", tag="idx", bufs=2)
                nc.sync.dma_start(idx[:], segq_d[i * P:(i + 1) * P, :])
                nc.gpsimd.indirect_dma_start(
                    out=xt[:], out_offset=None, in_=reps_d[:],
                    in_offset=bass.IndirectOffsetOnAxis(ap=idx[:, :1], axis=0))
                res = spool.tile([P, D], F32, name="res", tag="res", bufs=1)
                nc.sync.dma_start(res[:], resid_d[i * P:(i + 1) * P, :])
                nc.vector.tensor_add(xt[:], xt[:], res[:])
                x_own.append(xt)

            def rms_inv(src_ap, scratch_ap):
                ss = spool.tile([P, 1], F32, name="ss", tag="ss", bufs=2)
                nc.scalar.activation(scratch_ap, src_ap, AF.Square,
                                     bias=zb[:, :1], accum_out=ss[:])
                st = spool.tile([P, 1], F32, name="st", tag="st", bufs=2)
                nc.scalar.activation(st[:], ss[:], AF.Sqrt, scale=1.0 / D,
                                     bias=eb[:, :1])
                inv = spool.tile([P, 1], F32, name="inv", tag="inv", bufs=2)
                nc.vector.reciprocal_approx_fast(inv[:], st[:])
                return inv

            def alloc_ht(phase):
                return [hpool.tile([P, S], F16, name=f"ht{kt}_{phase}",
                                   tag=f"ht{kt}") for kt in range(KTD)]

            def ln_transpose_ag(phase, cc, ht):
                """LN chunk cc of x_own, transpose, AllGather, load into ht."""
                agin = dpool.tile([D, R * P], F16, name=f"agin_{phase}_{cc}",
                                  tag="agin", bufs=2 * NCHUNK)
                for j in range(R):
                    i = cc * R + j
                    h = spool.tile([P, D], F16, name="h", tag="h", bufs=2)
                    inv = rms_inv(x_own[i][:], h[:])
                    nc.vector.tensor_scalar_mul(h[:], x_own[i][:], inv[:, :1])
                    # 32x32 block transpose on DVE; the DMA permutes blocks
                    h32 = spool.tile([P, D], F16, name="h32", tag="h32",
                                     bufs=2)
                    nc.vector.transpose(h32[:], h[:])
                    for pi in range(P // 32):
                        col = j * P + pi * 32
                        nc.sync.dma_start(
                            agin[:, col:col + 32].rearrange(
                                "(cj ccc) r -> ccc cj r", ccc=32),
                            h32[pi * 32:(pi + 1) * 32, :].rearrange(
                                "p (cj rrr) -> p cj rrr", rrr=32))
                agout = dpool.tile([TP, D, R * P], F16,
                                   name=f"agout_{phase}_{cc}", tag="agout",
                                   bufs=2 * NCHUNK)
                nc.gpsimd.collective_compute(
                    "AllGather", ALU.bypass, replica_groups=GROUPS,
                    ins=[agin[:]], outs=[agout[:]])
                for kt in range(KTD):
                    for r in range(TP):
                        col = (cc * TP + r) * R * P
                        nc.sync.dma_start(
                            ht[kt][:, col:col + R * P],
                            agout[r, kt * P:(kt + 1) * P, :])

            def rs_chunk_add(rsin, phase, cc):
                """ReduceScatter chunk cc and add into own rows."""
                rsout = dpool.tile([R * P, D], F16, name=f"rso_{phase}_{cc}",
                                   tag="rsout", bufs=2 * NCHUNK)
                nc.gpsimd.collective_compute(
                    "ReduceScatter", ALU.add, replica_groups=GROUPS,
                    ins=[rsin[:]], outs=[rsout[:]])
                for j in range(R):
                    i = cc * R + j
                    rsl = spool.tile([P, D], F16, name="rsl", tag="rsl", bufs=2)
                    nc.sync.dma_start(rsl[:], rsout[j * P:(j + 1) * P, :])
                    nc.vector.tensor_add(x_own[i][:], x_own[i][:], rsl[:])

            # ---- initial AllGather for layer-0 attention ----
            ht = alloc_ht("a0")
            for cc in range(NCHUNK):
                ln_transpose_ag("a0", cc, ht)

            for l in range(L):
                # ---- attention weights ----
                wq = wpool.tile([P, KTD * OC], F16, name="wq", tag="wq")
                nc.sync.dma_start(wq[:], wq_d[l])
                wk = wpool.tile([P, KTD * OC], F16, name="wk", tag="wk")
                nc.sync.dma_start(wk[:], wk_d[l])
                wv = wpool.tile([P, KTD * OC], F16, name="wv", tag="wv")
                nc.sync.dma_start(wv[:], wv_d[l])
                wo = wpool.tile([P, NOPT * D], F16, name="wo", tag="wo")
                nc.sync.dma_start(wo[:], wo_d[l])

                # ---- Q/K (channel-major + RoPE) and V (row-major) ----
                qt = [bigpool.tile([P, S], F16, name=f"q{m}", tag=f"big{m}")
                      for m in range(NOPT)]
                ktt = [bigpool.tile([P, S], F16, name=f"k{m}",
                                    tag=f"big{NOPT + m}")
                       for m in range(NOPT)]
                vt = [vpool.tile([P, LH * 65], F16, name=f"v{rt}", tag=f"v{rt}")
                      for rt in range(NRA)]

                for sc in range(NQC):
                    ns = sc * QC
                    for wsb, outs in ((wq, qt), (wk, ktt)):
                        for m in range(NOPT):
                            pq = ppool.tile([P, QC], F32, name="pq", tag="proj",
                                            bufs=2, space="PSUM")
                            for kt in range(KTD):
                                nc.tensor.matmul(
                                    pq[:],
                                    lhsT=wsb[:, kt * OC + m * P:
                                             kt * OC + (m + 1) * P],
                                    rhs=ht[kt][:, ns:ns + QC],
                                    start=(kt == 0), stop=(kt == KTD - 1))
                            nc.scalar.copy(outs[m][:, ns:ns + QC], pq[:])
                        # RoPE on this column chunk
                        for t in outs:
                            rot = spool.tile([P, QC], F16, name="rot",
                                             tag="rot", bufs=2)
                            for blk in range(HPP):
                                o = blk * HD
                                hh = HD // 2
                                nc.sync.dma_start(rot[o:o + hh, :],
                                                  t[o + hh:o + HD, ns:ns + QC])
                                nc.sync.dma_start(rot[o + hh:o + HD, :],
                                                  t[o:o + hh, ns:ns + QC])
                            nc.vector.tensor_mul(t[:, ns:ns + QC],
                                                 t[:, ns:ns + QC],
                                                 cosb[:, ns:ns + QC])
                            nc.vector.tensor_mul(rot[:], rot[:],
                                                 sgnsin[:, ns:ns + QC])
                            nc.vector.tensor_add(t[:, ns:ns + QC],
                                                 t[:, ns:ns + QC], rot[:])
                    # V for the row-tiles of this chunk
                    for rt in range(sc * RPC, (sc + 1) * RPC):
                        v = vt[rt]
                        v3 = v[:].rearrange("p (h c) -> p h c", c=65)
                        nc.vector.memset(v3[:, :, 64:65], 1.0)
                        pv = ppool.tile([P, 512], F32, name="pv", tag="proj",
                                        bufs=2, space="PSUM")
                        for kt in range(KTD):
                            nc.tensor.matmul(
                                pv[:, :OC],
                                lhsT=ht[kt][:, rt * P:(rt + 1) * P],
                                rhs=wv[:, kt * OC:(kt + 1) * OC],
                                start=(kt == 0), stop=(kt == KTD - 1))
                        nc.scalar.copy(
                            v3[:, :, :64],
                            pv[:, :OC].rearrange("p (h c) -> p h c", c=64))

                # ---- flash-style causal attention + Wo + RS, per q-chunk ----
                ot = [bigpool.tile([P, S], F16, name=f"o{m}",
                                   tag=f"big{2 * NOPT + m}")
                      for m in range(NOPT)]
                htn = alloc_ht(f"f{l}")       # next-phase hT (FFN)
                for qc in range(NQC):
                    q0 = qc * QC
                    nkt = (q0 + QC) // P
                    sums = spool.tile([1, LH * QC], F32, name="sums",
                                      tag="sums", bufs=1)
                    for h in range(LH):
                        hp, ho = divmod(h * HD, P)
                        po = ppool.tile([P, QC], F32, name="po", tag="ot",
                                        bufs=2, space="PSUM")
                        for kt in range(nkt):
                            ps = ppool.tile([P, QC], F32, name="ps", tag="sc",
                                            bufs=3, space="PSUM")
                            nc.tensor.matmul(
                                ps[:],
                                lhsT=ktt[hp][ho:ho + HD, kt * P:(kt + 1) * P],
                                rhs=qt[hp][ho:ho + HD, q0:q0 + QC],
                                start=True, stop=True)
                            ex = spool.tile([P, QC], F16, name="ex", tag="ex",
                                            bufs=3)
                            nc.scalar.activation(ex[:], ps[:], AF.Exp,
                                                 bias=zb[:, :1], scale=ISCALE)
                            dd = kt - q0 // P
                            if dd >= 0:
                                nc.vector.tensor_mul(
                                    ex[:], ex[:],
                                    maskt[:, dd * QC:(dd + 1) * QC])
                            nc.tensor.matmul(
                                po[0:65, :],
                                lhsT=vt[kt][:, h * 65:(h + 1) * 65],
                                rhs=ex[:],
                                start=(kt == 0), stop=(kt == nkt - 1))
                        # unnormalized o' and the denominator row
                        nc.scalar.copy(ot[hp][ho:ho + HD, q0:q0 + QC],
                                       po[0:64, :])
                        nc.scalar.copy(sums[0:1, h * QC:(h + 1) * QC], po[64:65, :])
                    nc.vector.reciprocal_approx_fast(sums[:], sums[:])
                    for h in range(LH):
                        hp, ho = divmod(h * HD, P)
                        pb = ppool.tile([64, QC], F32, name="pb", tag="ot",
                                        bufs=2, space="PSUM")
                        nc.tensor.matmul(pb[:], lhsT=ones64[0:1, :],
                                         rhs=sums[0:1, h * QC:(h + 1) * QC],
                                         start=True, stop=True)
                        nc.vector.tensor_mul(ot[hp][ho:ho + HD, q0:q0 + QC],
                                             ot[hp][ho:ho + HD, q0:q0 + QC],
                                             pb[:])
                    # Wo for this chunk's row-tiles -> RS -> x += ; then the
                    # FFN-phase LN/transpose/AllGather for the same chunk.
                    rsin = dpool.tile([CW, D], F16, name=f"rsi_a{l}_{qc}",
                                      tag="rsin", bufs=2 * NCHUNK)
                    for rt in range(qc * RPC, (qc + 1) * RPC):
                        ow = spool.tile([P, D], F16, name="ow", tag="ow",
                                        bufs=2)
                        for (ds, dl) in d_chunks:
                            pw = ppool.tile([P, 512], F32, name="pw",
                                            tag="proj", bufs=2, space="PSUM")
                            for n in range(NOPT):
                                nc.tensor.matmul(
                                    pw[:, :dl],
                                    lhsT=ot[n][:, rt * P:(rt + 1) * P],
                                    rhs=wo[:, n * D + ds: n * D + ds + dl],
                                    start=(n == 0), stop=(n == NOPT - 1))
                            nc.vector.tensor_copy(ow[:, ds:ds + dl],
                                                  pw[:, :dl])
                        nc.sync.dma_start(
                            rsin[(rt - qc * RPC) * P:(rt - qc * RPC + 1) * P, :],
                            ow[:])
                    rs_chunk_add(rsin, f"a{l}", qc)
                    ln_transpose_ag(f"f{l}", qc, htn)
                ht = htn

                # ---- FFN ----
                wg = wpool.tile([P, KTD * FL], F16, name="wg", tag="wg")
                nc.sync.dma_start(wg[:], wg_d[l])
                wu = wpool.tile([P, KTD * FL], F16, name="wu", tag="wu")
                nc.sync.dma_start(wu[:], wu_d[l])
                wd = wpool.tile([P, KTF * D], F16, name="wd", tag="wd")
                nc.sync.dma_start(wd[:], wd_d[l])

                at = [bigpool.tile([P, S], F16, name=f"a{fm}", tag=f"big{fm}")
                      for fm in range(KTF)]
                if l < L - 1:
                    htn = alloc_ht(f"a{l + 1}")
                for sc in range(NQC):
                    ns = sc * QC
                    for fm in range(KTF):
                        pg = ppool.tile([P, QC], F32, name="pg", tag="proj",
                                        bufs=2, space="PSUM")
                        for kt in range(KTD):
                            nc.tensor.matmul(
                                pg[:],
                                lhsT=wg[:, kt * FL + fm * P:
                                        kt * FL + (fm + 1) * P],
                                rhs=ht[kt][:, ns:ns + QC],
                                start=(kt == 0), stop=(kt == KTD - 1))
                        pu = ppool.tile([P, QC], F32, name="pu", tag="sc",
                                        bufs=3, space="PSUM")
                        for kt in range(KTD):
                            nc.tensor.matmul(
                                pu[:],
                                lhsT=wu[:, kt * FL + fm * P:
                                        kt * FL + (fm + 1) * P],
                                rhs=ht[kt][:, ns:ns + QC],
                                start=(kt == 0), stop=(kt == KTD - 1))
                        sg = spool.tile([P, QC], F16, name="sg", tag="sg",
                                        bufs=2)
                        nc.scalar.activation(sg[:], pg[:], AF.Sigmoid,
                                             bias=zb[:, :1])
                        nc.vector.tensor_mul(sg[:], sg[:], pg[:])
                        nc.vector.tensor_mul(at[fm][:, ns:ns + QC], sg[:],
                                             pu[:])
                    # Wd for this chunk -> RS -> x += ; then next-phase AG
                    rsin2 = dpool.tile([CW, D], F16, name=f"rsi_f{l}_{sc}",
                                       tag="rsin", bufs=2 * NCHUNK)
                    for rt in range(sc * RPC, (sc + 1) * RPC):
                        dw = spool.tile([P, D], F16, name="dw", tag="ow",
                                        bufs=2)
                        for (ds, dl) in d_chunks:
                            pd = ppool.tile([P, 512], F32, name="pd",
                                            tag="proj", bufs=2, space="PSUM")
                            for kt in range(KTF):
                                nc.tensor.matmul(
                                    pd[:, :dl],
                                    lhsT=at[kt][:, rt * P:(rt + 1) * P],
                                    rhs=wd[:, kt * D + ds: kt * D + ds + dl],
                                    start=(kt == 0), stop=(kt == KTF - 1))
                            nc.vector.tensor_copy(dw[:, ds:ds + dl],
                                                  pd[:, :dl])
                        nc.sync.dma_start(
                            rsin2[(rt - sc * RPC) * P:
                                  (rt - sc * RPC + 1) * P, :],
                            dw[:])
                    rs_chunk_add(rsin2, f"f{l}", sc)
                    if l < L - 1:
                        ln_transpose_ag(f"a{l + 1}", sc, htn)
                    else:
                        # final RMS norm * fnorm for this chunk's rows
                        for j in range(R):
                            i = sc * R + j
                            fo = spool.tile([P, D], F32, name="fo", tag="fo",
                                            bufs=1)
                            inv = rms_inv(x_own[i][:], fo[:])
                            nc.vector.tensor_scalar_mul(fo[:], x_own[i][:],
                                                        inv[:, :1])
                            nc.vector.tensor_mul(fo[:], fo[:], fnormb[:])
                            nc.sync.dma_start(out_d[i * P:(i + 1) * P, :],
                                              fo[:])
                if l < L - 1:
                    ht = htn

    nc.compile()
    return nc


# --------------------------------------------------------------------------
# host-side input preparation
# --------------------------------------------------------------------------

def _own_rowtiles(c, tp):
    """Global row-tile indices owned by rank tp, in local order."""
    return [(cc * TP + tp) * c["R"] + j
            for cc in range(c["NCHUNK"]) for j in range(c["R"])]


def make_in_maps(cfg, inputs):
    c = _derive(cfg)
    B, S, D, K, H, HD, F, L = (c[k] for k in ("B", "S", "D", "K", "H", "HD", "F", "L"))
    LH, OC, NOPT, OWN = c["LH"], c["OC"], c["NOPT"], c["OWN"]
    KTD, FL, KTF = c["KTD"], c["FL"], c["KTF"]

    gi = {k: np.asarray(v) for k, v in inputs.items() if k != "seq_len"}
    x_processed = gi["x_processed"].astype(np.float32)
    boundaries = gi["boundaries"].astype(np.int64)
    counts = gi["counts"].astype(np.int64)
    x_residual = gi["x_residual"].astype(np.float32)
    cos = gi["cos"].astype(np.float32)
    sin = gi["sin"].astype(np.float32)
    start_emb = gi["start_emb"].astype(np.float32)
    ln1 = gi["ln1"].astype(np.float32)
    ln2 = gi["ln2"].astype(np.float32)
    fnorm = gi["fnorm"].astype(np.float32)
    Wq, Wk, Wv, Wo = (gi[k].astype(np.float32) for k in ("Wq", "Wk", "Wv", "Wo"))
    Wg, Wu, Wd = (gi[k].astype(np.float32) for k in ("Wg", "Wu", "Wd"))

    # segment index per position (searchsorted over masked boundaries)
    seg = np.empty((B, S), np.int32)
    for b in range(B):
        bnd = np.where(np.arange(K) < counts[b], boundaries[b], S)
        seg[b] = np.searchsorted(bnd, np.arange(S), side="left").astype(np.int32)

    pidx = np.arange(P)
    cosb = cos.T[pidx % HD].astype(np.float16)                       # [P, S]
    sgn = np.where((pidx % HD) < HD // 2, -1.0, 1.0)[:, None]
    sgnsin = (sin.T[pidx % HD] * sgn).astype(np.float16)             # [P, S]
    fnormb = np.broadcast_to(fnorm, (P, D)).astype(np.float16).copy()
    QCc, RPCc = c["QC"], c["CW"] // P
    kk = np.arange(P)[:, None]
    qq = np.arange(QCc)[None, :]
    masks = np.concatenate(
        [(dd * P + kk <= qq).astype(np.float16) for dd in range(RPCc)],
        axis=1)

    def kt_layout(w):      # [D or FL, C] -> [P, KT*C]
        n, cdim = w.shape
        return np.ascontiguousarray(
            w.reshape(n // P, P, cdim).transpose(1, 0, 2).reshape(P, -1)
        ).astype(np.float16)

    in_maps = []
    for cid in range(NCORES):
        b, tp = divmod(cid, TP)
        gts = _own_rowtiles(c, tp)
        rows = np.concatenate([np.arange(g * P, (g + 1) * P) for g in gts])
        hb = tp * OC
        fb = tp * FL
        reps = np.concatenate([start_emb[None], x_processed[b]], axis=0)
        m = dict(
            reps=np.ascontiguousarray(reps, np.float32),
            segq=np.ascontiguousarray(seg[b, rows].reshape(OWN, 1)),
            resid=np.ascontiguousarray(x_residual[b, rows]),
            cosb=cosb, sgnsin=sgnsin, fnormb=fnormb, masks=masks,
            wq=np.stack([kt_layout(ln1[l][:, None] * Wq[l][:, hb:hb + OC])
                         for l in range(L)]),
            wk=np.stack([kt_layout(ln1[l][:, None] * Wk[l][:, hb:hb + OC])
                         for l in range(L)]),
            wv=np.stack([kt_layout(ln1[l][:, None] * Wv[l][:, hb:hb + OC])
                         for l in range(L)]),
            wo=np.stack([kt_layout(Wo[l][hb:hb + OC, :]) for l in range(L)]),
            wg=np.stack([kt_layout(ln2[l][:, None] * Wg[l][:, fb:fb + FL])
                         for l in range(L)]),
            wu=np.stack([kt_layout(ln2[l][:, None] * Wu[l][:, fb:fb + FL])
                         for l in range(L)]),
            wd=np.stack([kt_layout(Wd[l][fb:fb + FL, :]) for l in range(L)]),
        )
        in_maps.append(m)
    return in_maps


def assemble_output(cfg, results):
    c = _derive(cfg)
    B, S, D, OWN = c["B"], c["S"], c["D"], c["OWN"]
    out = np.empty((B, S, D), np.float32)
    for cid in range(NCORES):
        b, tp = divmod(cid, TP)
        gts = _own_rowtiles(c, tp)
        r = results[cid]["out"]
        for i, g in enumerate(gts):
            out[b, g * P:(g + 1) * P] = r[i * P:(i + 1) * P]
    return out


_NC_CACHE = {}


def _get_nc(cfg):
    key = tuple(sorted(cfg.items()))
    if key not in _NC_CACHE:
        _NC_CACHE[key] = build_nc(cfg)
    return _NC_CACHE[key]


def kernel(**inputs) -> np.ndarray:
    cfg = FULL_CFG
    nc = _get_nc(cfg)
    in_maps = make_in_maps(cfg, inputs)
    res = bass_utils.run_bass_kernel_spmd(nc, in_maps,
                                          core_ids=list(range(NCORES)))
    return assemble_output(cfg, res.results)
